# revision 1
# baseline (speedup 1.0000x reference)
"""BACENET gnn_message_passing kernel for 8 TRN2 NeuronCores.

Strategy: sort pairs by first_atom_idx on host; atoms grouped into
128-atom windows; each window's pairs padded to a fixed number of
128-pair blocks (NBW, same for every window so all cores run one SPMD
graph).  On device, per block: build the angular monomials with wide
vector ops, form data[p, r*34+l] = radial[p,r]*ang[p,l] with one
broadcast tensor_tensor, build the one-hot scatter matrix with one
tensor_scalar(is_equal) against an iota tile, and accumulate
onehot.T @ data into PSUM with the TensorEngine (the segment sum).
Epilogue per window: square (ScalarE), contract the lambda weights
(broadcast multiply + reduce on VectorE), DMA out.
"""

import math
import numpy as np

TRACE = False          # test harness can set kernel.TRACE = True for profiling
LAST_RESULT = None

NAT = 12500
NPAIRS = 250000
NRAD = 16
L = 34
NLAM = 4
NCORE = 8
AW = 128                      # atoms per window
NWINTOT = (NAT + AW - 1) // AW        # 98
NWIN = (NWINTOT + NCORE - 1) // NCORE  # 13 windows per core
FEATC = 20                    # 16 radial + 3 unit vec + 1 local idx


def _graded_order(zeta):
    """Monomial order produced by the on-device recurrence.

    deg1 = [z, y, x]; deg s = [z*deg(s-1)] + [y*(lz==0 sublist of s-1)] + [x*x^(s-1)].
    Returns list of (lx,ly,lz) triples in that order.
    """
    deg = [[(0, 0, 1), (0, 1, 0), (1, 0, 0)]]
    for s in range(2, zeta + 1):
        prev = deg[-1]
        lz0 = [t for t in prev if t[2] == 0]
        cur = ([(a, b, c + 1) for (a, b, c) in prev]
               + [(a, b + 1, 0) for (a, b, _) in lz0]
               + [(prev[-1][0] + 1, 0, 0)])
        cur[-1] = (s, 0, 0)
        deg.append(cur)
    out = []
    for d in deg:
        out.extend(d)
    return out


def _build_graph(nblk, nbw, lx, ly, lz, graded):
    """Build the SPMD Bass graph. Returns (nc, out_name)."""
    import concourse.bass as bass
    import concourse.bacc as bacc
    import concourse.mybir as mybir
    from concourse import tile

    dt = mybir.dt.float32
    Alu = mybir.AluOpType
    Act = mybir.ActivationFunctionType

    nc = bacc.Bacc("TRN2", target_bir_lowering=False, debug=False,
                   num_devices=NCORE)

    feat_d = nc.dram_tensor("feat", [128, nblk, FEATC], dt, kind="ExternalInput")
    iota_d = nc.dram_tensor("iota", [128, 128], dt, kind="ExternalInput")
    w4_d = nc.dram_tensor("w4", [128, NLAM * NRAD * L], dt, kind="ExternalInput")
    out_d = nc.dram_tensor("out", [NWIN * 128, NLAM * NRAD], dt,
                           kind="ExternalOutput")

    W4C = NLAM * NRAD * L  # 2176

    with tile.TileContext(nc) as tc:
        with (
            tc.tile_pool(name="const", bufs=1) as cpool,
            tc.tile_pool(name="work", bufs=3) as pool,
            tc.tile_pool(name="blk", bufs=4) as bpool,
            tc.tile_pool(name="psum", bufs=2, space="PSUM") as psum,
        ):
            iota = cpool.tile([128, 128], dt, tag="iota")
            nc.sync.dma_start(iota[:], iota_d[:])
            w4 = cpool.tile([128, W4C], dt, tag="w4")
            nc.sync.dma_start(w4[:], w4_d[:])

            for w in range(NWIN):
                feat = pool.tile([128, nbw, FEATC], dt, tag="feat")
                nc.sync.dma_start(feat[:], feat_d[:, w * nbw:(w + 1) * nbw, :])

                mono = pool.tile([128, L, nbw], dt, tag="mono")
                # unit-vector columns with +1e-12, like the reference
                for c in range(3):
                    # deg1 order [z, y, x] -> mono rows 0,1,2
                    nc.vector.tensor_scalar(
                        mono[:, c, :], feat[:, :, 18 - c], 1e-12, None, Alu.add)
                if graded:
                    # graded recurrence: 3 contiguous wide ops per degree
                    sizes = [3]
                    zeta = 1
                    while sum(sizes) < L:
                        zeta += 1
                        sizes.append(sizes[-1] + zeta + 1)
                    offs = [0]
                    for sz in sizes[:-1]:
                        offs.append(offs[-1] + sz)
                    # offs[i] = start of degree i+1 block, sizes[i] = its len
                    for s in range(2, zeta + 1):
                        o_prev, t_prev = offs[s - 2], sizes[s - 2]
                        o_cur = offs[s - 1]
                        uz = feat[:, :, 18].unsqueeze(1).broadcast_to(
                            [128, t_prev, nbw])
                        nc.vector.tensor_tensor(
                            mono[:, o_cur:o_cur + t_prev, :],
                            mono[:, o_prev:o_prev + t_prev, :], uz, Alu.mult)
                        # lz==0 sublist of degree s-1 = last s entries
                        o_lz0 = o_prev + t_prev - s
                        uy = feat[:, :, 17].unsqueeze(1).broadcast_to(
                            [128, s, nbw])
                        nc.vector.tensor_tensor(
                            mono[:, o_cur + t_prev:o_cur + t_prev + s, :],
                            mono[:, o_lz0:o_lz0 + s, :], uy, Alu.mult)
                        ux = feat[:, :, 16].unsqueeze(1).broadcast_to(
                            [128, 1, nbw])
                        nc.vector.tensor_tensor(
                            mono[:, o_cur + t_prev + s:o_cur + t_prev + s + 1, :],
                            mono[:, o_prev + t_prev - 1:o_prev + t_prev, :],
                            ux, Alu.mult)
                else:
                    # generic: powers 1..4 per component then per-l products
                    pow_t = pool.tile([128, 3, 5, nbw], dt, tag="pow")
                    for c, base in ((0, 16), (1, 17), (2, 18)):
                        nc.vector.tensor_scalar(
                            pow_t[:, c, 1, :], feat[:, :, base], 1e-12, None,
                            Alu.add)
                        for e in range(2, 5):
                            nc.vector.tensor_tensor(
                                pow_t[:, c, e, :], pow_t[:, c, e - 1, :],
                                pow_t[:, c, 1, :], Alu.mult)
                    for li in range(L):
                        exps = [(0, lx[li]), (1, ly[li]), (2, lz[li])]
                        exps = [(c, e) for c, e in exps if e > 0]
                        if not exps:
                            nc.vector.memset(mono[:, li, :], 1.0)
                            continue
                        c0, e0 = exps[0]
                        if len(exps) == 1:
                            nc.vector.tensor_copy(mono[:, li, :],
                                                  pow_t[:, c0, e0, :])
                        else:
                            c1, e1 = exps[1]
                            nc.vector.tensor_tensor(
                                mono[:, li, :], pow_t[:, c0, e0, :],
                                pow_t[:, c1, e1, :], Alu.mult)
                            if len(exps) == 3:
                                c2, e2 = exps[2]
                                nc.vector.tensor_tensor(
                                    mono[:, li, :], mono[:, li, :],
                                    pow_t[:, c2, e2, :], Alu.mult)

                psA = psum.tile([128, 8 * L], dt, tag="psA")
                psB = psum.tile([128, 8 * L], dt, tag="psB")
                for b in range(nbw):
                    data = bpool.tile([128, NRAD, L], dt, tag="data")
                    rad = feat[:, b, 0:16].unsqueeze(2).broadcast_to(
                        [128, NRAD, L])
                    ang = mono[:, :, b].unsqueeze(1).broadcast_to(
                        [128, NRAD, L])
                    deng = nc.gpsimd if (b % 3 == 2) else nc.vector
                    deng.tensor_tensor(data[:], rad, ang, Alu.mult)

                    oh = bpool.tile([128, 128], dt, tag="oh")
                    oeng = nc.vector if (b % 3 == 2) else nc.gpsimd
                    oeng.tensor_scalar(
                        oh[:], iota[:], feat[:, b, 19:20], None, Alu.is_equal)

                    d2 = data.rearrange("p r l -> p (r l)")
                    nc.tensor.matmul(psA[:], oh[:], d2[:, 0:8 * L],
                                     start=(b == 0), stop=(b == nbw - 1))
                    nc.tensor.matmul(psB[:], oh[:], d2[:, 8 * L:16 * L],
                                     start=(b == 0), stop=(b == nbw - 1))

                g2 = pool.tile([128, NRAD * L], dt, tag="g2")
                nc.scalar.activation(g2[:, 0:8 * L], psA[:], Act.Square)
                nc.scalar.activation(g2[:, 8 * L:16 * L], psB[:], Act.Square)

                prod = pool.tile([128, W4C], dt, tag="prod")
                g2b = g2.unsqueeze(1).broadcast_to([128, NLAM, NRAD * L])
                w4v = w4.rearrange("p (z q) -> p z q", z=NLAM)
                prodv = prod.rearrange("p (z q) -> p z q", z=NLAM)
                nc.vector.tensor_tensor(prodv, g2b, w4v, Alu.mult)

                ow = pool.tile([128, NLAM * NRAD], dt, tag="ow")
                nc.vector.tensor_reduce(
                    ow[:], prod.rearrange("p (q l) -> p q l", l=L),
                    mybir.AxisListType.X, Alu.add)
                nc.sync.dma_start(out_d[w * 128:(w + 1) * 128, :], ow[:])

    return nc


def kernel(**inputs):
    z = int(inputs["z"])
    rij_unit = np.asarray(inputs["rij_unit"], np.float32)
    radial_ij = np.asarray(inputs["radial_ij"], np.float32)
    first_atom_idx = np.asarray(inputs["first_atom_idx"], np.int32)
    lambda_weights = np.asarray(inputs["lambda_weights"], np.float32)
    lxlylz = np.asarray(inputs["lxlylz"], np.int32)
    lxlylz_sum = np.asarray(inputs["lxlylz_sum"], np.int32)
    fact_norm = np.asarray(inputs["fact_norm"], np.float32)
    nat = int(inputs["nat"])

    npairs = rij_unit.shape[0]
    nwintot = (nat + AW - 1) // AW
    assert nwintot <= NWIN * NCORE

    # ---- host: sort pairs by atom, window them, pack into fixed slots ----
    # Each of the NCORE*NWIN slots holds nbw 128-pair blocks for ONE
    # 128-atom window; a window with more pairs than one slot holds is
    # split across several slots and the host adds the partial outputs.
    order = np.argsort(first_atom_idx, kind="stable")
    sidx = first_atom_idx[order]
    wg = sidx // AW                               # window of each sorted pair
    nslots = NWIN * NCORE
    win_counts = np.bincount(wg, minlength=nwintot)
    win_start = np.concatenate([[0], np.cumsum(win_counts)[:-1]])
    bw = (win_counts + 127) // 128                # blocks needed per window
    nbw = max(1, int(np.ceil(bw.sum() / nslots)))
    while int(np.sum(np.maximum((bw + nbw - 1) // nbw, 1))) > nslots:
        nbw += 1
    nblk = NWIN * nbw

    slots_per_win = np.maximum((bw + nbw - 1) // nbw, 1)
    slot0_of_win = np.concatenate([[0], np.cumsum(slots_per_win)[:-1]])
    slot_window = np.full(nslots, -1, np.int64)   # slot -> window id
    for wid in range(nwintot):
        for k in range(slots_per_win[wid]):
            slot_window[slot0_of_win[wid] + k] = wid

    rank = np.arange(npairs) - win_start[wg]      # rank within own window
    slot = slot0_of_win[wg] + rank // (nbw * 128)
    r2 = rank % (nbw * 128)
    nb = r2 // 128
    pp = r2 % 128
    core = slot // NWIN
    col = (slot % NWIN) * nbw + nb

    feat = np.zeros((NCORE, 128, nblk, FEATC), np.float32)
    feat[core, pp, col, 0:16] = radial_ij[order]
    feat[core, pp, col, 16:19] = rij_unit[order]
    feat[core, pp, col, 19] = (sidx - wg * AW).astype(np.float32)

    # ---- monomial order on device ----
    ltrip = [tuple(t) for t in lxlylz.tolist()]
    graded_ref = _graded_order(4) if len(ltrip) == L else None
    graded = graded_ref is not None and sorted(ltrip) == sorted(graded_ref)
    if graded:
        # device computes graded order; permute W columns to match
        pos = {}
        for i, t in enumerate(ltrip):
            pos.setdefault(t, []).append(i)
        perm = []
        for t in graded_ref:
            perm.append(pos[t].pop(0))
        perm = np.array(perm, np.int32)           # device l -> input l
    else:
        perm = np.arange(len(ltrip), dtype=np.int32)

    lam = lambda_weights[:, None] ** lxlylz_sum.astype(np.float32)[None, :]
    wrow = lam * fact_norm[None, :] * (2.0 ** (1.0 - float(z)))   # [NLAM, L]
    wrow = wrow[:, perm]                          # reorder to device order
    w4 = np.tile(wrow[:, None, :], (1, NRAD, 1)).reshape(-1)      # (z, r, l)
    w4_t = np.tile(w4[None, :], (128, 1)).astype(np.float32)

    iota_t = np.tile(np.arange(128, dtype=np.float32)[None, :], (128, 1))

    lx, ly, lz_ = (lxlylz[:, 0].tolist(), lxlylz[:, 1].tolist(),
                   lxlylz[:, 2].tolist())

    nc = _build_graph(nblk, nbw, lx, ly, lz_, graded)
    nc.compile()

    from concourse.bass_utils import run_bass_kernel_spmd
    in_maps = [{"feat": feat[i], "iota": iota_t, "w4": w4_t}
               for i in range(NCORE)]
    global LAST_RESULT
    res = run_bass_kernel_spmd(nc, in_maps, core_ids=list(range(NCORE)),
                               trace=TRACE)
    LAST_RESULT = res

    # ---- host: unshard (accumulate split-window slots) ----
    acc = np.zeros((nwintot * AW, NLAM * NRAD), np.float32)
    for s in range(nslots):
        wid = slot_window[s]
        if wid < 0:
            continue
        part = res.results[s // NWIN]["out"]
        lw = s % NWIN
        acc[wid * AW:(wid + 1) * AW] += part[lw * 128:(lw + 1) * 128]
    out = acc.reshape(nwintot * AW, NLAM, NRAD)[:nat]
    return np.ascontiguousarray(out.transpose(0, 2, 1))   # [nat, NRAD, NLAM]



# revision 2
# speedup vs baseline: 2.2803x; 2.2803x over previous
"""BACENET gnn_message_passing kernel for 8 TRN2 NeuronCores.

Strategy: sort pairs by first_atom_idx on host; atoms grouped into
128-atom windows; each window's pairs padded to a fixed number of
128-pair blocks (nbw, same for every window so all cores run one SPMD
graph).  The scatter one-hot matrices are precomputed on the HOST in
bf16 and DMA'd in (the on-device is_equal build was the bottleneck).
On device, per block: build the angular monomials with wide vector ops
(fp32), form data[p, r*34+l] = radial[p,r]*ang[p,l] with one broadcast
tensor_tensor into a bf16 tile (split across Vector/GpSimd), and
accumulate onehot.T @ data into PSUM with bf16 TensorEngine matmuls
(the segment sum).  Epilogue per window: square (ScalarE), contract
the lambda weights (broadcast multiply + reduce on VectorE), DMA out.
"""

import math
import numpy as np

TRACE = False          # test harness can set kernel.TRACE = True for profiling
LAST_RESULT = None

NAT = 12500
NPAIRS = 250000
NRAD = 16
L = 34
NLAM = 4
NCORE = 8
AW = 128                      # atoms per window
NWINTOT = (NAT + AW - 1) // AW        # 98
NWIN = (NWINTOT + NCORE - 1) // NCORE  # 13 windows per core
FEATC = 20                    # 16 radial + 3 unit vec + 1 local idx


def _graded_order(zeta):
    """Monomial order produced by the on-device recurrence.

    deg1 = [z, y, x]; deg s = [z*deg(s-1)] + [y*(lz==0 sublist of s-1)] + [x*x^(s-1)].
    Returns list of (lx,ly,lz) triples in that order.
    """
    deg = [[(0, 0, 1), (0, 1, 0), (1, 0, 0)]]
    for s in range(2, zeta + 1):
        prev = deg[-1]
        lz0 = [t for t in prev if t[2] == 0]
        cur = ([(a, b, c + 1) for (a, b, c) in prev]
               + [(a, b + 1, 0) for (a, b, _) in lz0]
               + [(prev[-1][0] + 1, 0, 0)])
        cur[-1] = (s, 0, 0)
        deg.append(cur)
    out = []
    for d in deg:
        out.extend(d)
    return out


def _build_graph(nblk, nbw, lx, ly, lz, graded):
    """Build the SPMD Bass graph. Returns nc."""
    import concourse.bass as bass
    import concourse.bacc as bacc
    import concourse.mybir as mybir
    from concourse import tile

    dt = mybir.dt.float32
    bf = mybir.dt.bfloat16
    Alu = mybir.AluOpType
    Act = mybir.ActivationFunctionType

    nc = bacc.Bacc("TRN2", target_bir_lowering=False, debug=False,
                   num_devices=NCORE)

    feat_d = nc.dram_tensor("feat", [128, nblk, FEATC], dt, kind="ExternalInput")
    oh_d = nc.dram_tensor("oh", [128, nblk * 128], bf, kind="ExternalInput")
    w4_d = nc.dram_tensor("w4", [128, NLAM * NRAD * L], bf, kind="ExternalInput")
    out_d = nc.dram_tensor("out", [NWIN * 128, NLAM * NRAD], dt,
                           kind="ExternalOutput")

    W4C = NLAM * NRAD * L  # 2176
    HALF = 8 * L           # 272

    with tile.TileContext(nc) as tc:
        with (
            tc.tile_pool(name="const", bufs=1) as cpool,
            tc.tile_pool(name="work", bufs=2) as pool,
            tc.tile_pool(name="blk", bufs=4) as bpool,
            tc.tile_pool(name="psum", bufs=2, space="PSUM") as psum,
        ):
            w4 = cpool.tile([128, W4C], bf, tag="w4")
            nc.sync.dma_start(w4[:], w4_d[:])

            for w in range(NWIN):
                feat = pool.tile([128, nbw, FEATC], dt, tag="feat")
                nc.sync.dma_start(feat[:], feat_d[:, w * nbw:(w + 1) * nbw, :])
                ohw = pool.tile([128, nbw * 128], bf, tag="ohw")
                nc.sync.dma_start(
                    ohw[:], oh_d[:, w * nbw * 128:(w + 1) * nbw * 128])

                # monomials, layout [128, nbw, L] so l is contiguous
                mono = pool.tile([128, nbw, L], dt, tag="mono")
                for c in range(3):
                    # deg1 order [z, y, x] -> mono cols 0,1,2
                    nc.vector.tensor_scalar(
                        mono[:, :, c], feat[:, :, 18 - c], 1e-12, None, Alu.add)
                if graded:
                    # graded recurrence: 3 contiguous wide ops per degree
                    sizes = [3]
                    zeta = 1
                    while sum(sizes) < L:
                        zeta += 1
                        sizes.append(sizes[-1] + zeta + 1)
                    offs = [0]
                    for sz in sizes[:-1]:
                        offs.append(offs[-1] + sz)
                    for s in range(2, zeta + 1):
                        o_prev, t_prev = offs[s - 2], sizes[s - 2]
                        o_cur = offs[s - 1]
                        uz = feat[:, :, 18].unsqueeze(2).broadcast_to(
                            [128, nbw, t_prev])
                        nc.vector.tensor_tensor(
                            mono[:, :, o_cur:o_cur + t_prev],
                            mono[:, :, o_prev:o_prev + t_prev], uz, Alu.mult)
                        # lz==0 sublist of degree s-1 = last s entries
                        o_lz0 = o_prev + t_prev - s
                        uy = feat[:, :, 17].unsqueeze(2).broadcast_to(
                            [128, nbw, s])
                        nc.vector.tensor_tensor(
                            mono[:, :, o_cur + t_prev:o_cur + t_prev + s],
                            mono[:, :, o_lz0:o_lz0 + s], uy, Alu.mult)
                        ux = feat[:, :, 16].unsqueeze(2).broadcast_to(
                            [128, nbw, 1])
                        nc.vector.tensor_tensor(
                            mono[:, :, o_cur + t_prev + s:o_cur + t_prev + s + 1],
                            mono[:, :, o_prev + t_prev - 1:o_prev + t_prev],
                            ux, Alu.mult)
                else:
                    pow_t = pool.tile([128, 3, 5, nbw], dt, tag="pow")
                    for c, base in ((0, 16), (1, 17), (2, 18)):
                        nc.vector.tensor_scalar(
                            pow_t[:, c, 1, :], feat[:, :, base], 1e-12, None,
                            Alu.add)
                        for e in range(2, 5):
                            nc.vector.tensor_tensor(
                                pow_t[:, c, e, :], pow_t[:, c, e - 1, :],
                                pow_t[:, c, 1, :], Alu.mult)
                    for li in range(L):
                        exps = [(0, lx[li]), (1, ly[li]), (2, lz[li])]
                        exps = [(c, e) for c, e in exps if e > 0]
                        if not exps:
                            nc.vector.memset(mono[:, :, li], 1.0)
                            continue
                        c0, e0 = exps[0]
                        if len(exps) == 1:
                            nc.vector.tensor_copy(mono[:, :, li],
                                                  pow_t[:, c0, e0, :])
                        else:
                            c1, e1 = exps[1]
                            nc.vector.tensor_tensor(
                                mono[:, :, li], pow_t[:, c0, e0, :],
                                pow_t[:, c1, e1, :], Alu.mult)
                            if len(exps) == 3:
                                c2, e2 = exps[2]
                                nc.vector.tensor_tensor(
                                    mono[:, :, li], mono[:, :, li],
                                    pow_t[:, c2, e2, :], Alu.mult)

                psA = psum.tile([128, HALF], dt, tag="psA")
                psB = psum.tile([128, HALF], dt, tag="psB")
                for b in range(nbw):
                    data = bpool.tile([128, NRAD, L], bf, tag="data")
                    rad = feat[:, b, 0:16].unsqueeze(2).broadcast_to(
                        [128, NRAD, L])
                    ang = mono[:, b, :].unsqueeze(1).broadcast_to(
                        [128, NRAD, L])
                    deng = nc.gpsimd if (b % 2 == 1) else nc.vector
                    deng.tensor_tensor(data[:], rad, ang, Alu.mult)

                    d2 = data.rearrange("p r l -> p (r l)")
                    ohb = ohw[:, b * 128:(b + 1) * 128]
                    nc.tensor.matmul(psA[:], ohb, d2[:, 0:HALF],
                                     start=(b == 0), stop=(b == nbw - 1))
                    nc.tensor.matmul(psB[:], ohb, d2[:, HALF:2 * HALF],
                                     start=(b == 0), stop=(b == nbw - 1))

                g2 = pool.tile([128, NRAD * L], bf, tag="g2")
                nc.scalar.activation(g2[:, 0:HALF], psA[:], Act.Square)
                nc.scalar.activation(g2[:, HALF:2 * HALF], psB[:], Act.Square)

                prod = pool.tile([128, W4C], bf, tag="prod")
                g2b = g2.unsqueeze(1).broadcast_to([128, NLAM, NRAD * L])
                w4v = w4.rearrange("p (z q) -> p z q", z=NLAM)
                prodv = prod.rearrange("p (z q) -> p z q", z=NLAM)
                nc.vector.tensor_tensor(prodv, g2b, w4v, Alu.mult)

                ow = pool.tile([128, NLAM * NRAD], dt, tag="ow")
                nc.vector.tensor_reduce(
                    ow[:], prod.rearrange("p (q l) -> p q l", l=L),
                    mybir.AxisListType.X, Alu.add)
                nc.sync.dma_start(out_d[w * 128:(w + 1) * 128, :], ow[:])

    return nc


def kernel(**inputs):
    import ml_dtypes

    z = int(inputs["z"])
    rij_unit = np.asarray(inputs["rij_unit"], np.float32)
    radial_ij = np.asarray(inputs["radial_ij"], np.float32)
    first_atom_idx = np.asarray(inputs["first_atom_idx"], np.int32)
    lambda_weights = np.asarray(inputs["lambda_weights"], np.float32)
    lxlylz = np.asarray(inputs["lxlylz"], np.int32)
    lxlylz_sum = np.asarray(inputs["lxlylz_sum"], np.int32)
    fact_norm = np.asarray(inputs["fact_norm"], np.float32)
    nat = int(inputs["nat"])

    npairs = rij_unit.shape[0]
    nwintot = (nat + AW - 1) // AW
    assert nwintot <= NWIN * NCORE

    # ---- host: sort pairs by atom, window them, pack into fixed slots ----
    order = np.argsort(first_atom_idx, kind="stable")
    sidx = first_atom_idx[order]
    wg = sidx // AW                               # window of each sorted pair
    nslots = NWIN * NCORE
    win_counts = np.bincount(wg, minlength=nwintot)
    win_start = np.concatenate([[0], np.cumsum(win_counts)[:-1]])
    bw = (win_counts + 127) // 128                # blocks needed per window
    nbw = max(1, int(np.ceil(bw.sum() / nslots)))
    while int(np.sum(np.maximum((bw + nbw - 1) // nbw, 1))) > nslots:
        nbw += 1
    nblk = NWIN * nbw

    slots_per_win = np.maximum((bw + nbw - 1) // nbw, 1)
    slot0_of_win = np.concatenate([[0], np.cumsum(slots_per_win)[:-1]])
    slot_window = np.full(nslots, -1, np.int64)   # slot -> window id
    for wid in range(nwintot):
        for k in range(slots_per_win[wid]):
            slot_window[slot0_of_win[wid] + k] = wid

    rank = np.arange(npairs) - win_start[wg]      # rank within own window
    slot = slot0_of_win[wg] + rank // (nbw * 128)
    r2 = rank % (nbw * 128)
    nb = r2 // 128
    pp = r2 % 128
    core = slot // NWIN
    col = (slot % NWIN) * nbw + nb

    feat = np.zeros((NCORE, 128, nblk, FEATC), np.float32)
    feat[core, pp, col, 0:16] = radial_ij[order]
    feat[core, pp, col, 16:19] = rij_unit[order]
    lidx = (sidx - wg * AW).astype(np.int64)
    feat[core, pp, col, 19] = lidx.astype(np.float32)

    # host-precomputed scatter one-hot (bf16): oh[p, b*128 + a] = 1 when the
    # pair in partition p of block b belongs to local atom a of its window
    oh = np.zeros((NCORE, 128, nblk * 128), ml_dtypes.bfloat16)
    oh[core, pp, col * 128 + lidx] = 1.0

    # ---- monomial order on device ----
    ltrip = [tuple(t) for t in lxlylz.tolist()]
    graded_ref = _graded_order(4) if len(ltrip) == L else None
    graded = graded_ref is not None and sorted(ltrip) == sorted(graded_ref)
    if graded:
        pos = {}
        for i, t in enumerate(ltrip):
            pos.setdefault(t, []).append(i)
        perm = []
        for t in graded_ref:
            perm.append(pos[t].pop(0))
        perm = np.array(perm, np.int32)           # device l -> input l
    else:
        perm = np.arange(len(ltrip), dtype=np.int32)

    lam = lambda_weights[:, None] ** lxlylz_sum.astype(np.float32)[None, :]
    wrow = lam * fact_norm[None, :] * (2.0 ** (1.0 - float(z)))   # [NLAM, L]
    wrow = wrow[:, perm]                          # reorder to device order
    w4 = np.tile(wrow[:, None, :], (1, NRAD, 1)).reshape(-1)      # (z, r, l)
    w4_t = np.tile(w4[None, :], (128, 1)).astype(ml_dtypes.bfloat16)

    lx, ly, lz_ = (lxlylz[:, 0].tolist(), lxlylz[:, 1].tolist(),
                   lxlylz[:, 2].tolist())

    nc = _build_graph(nblk, nbw, lx, ly, lz_, graded)
    nc.compile()

    from concourse.bass_utils import run_bass_kernel_spmd
    in_maps = [{"feat": feat[i], "oh": oh[i], "w4": w4_t}
               for i in range(NCORE)]
    global LAST_RESULT
    res = run_bass_kernel_spmd(nc, in_maps, core_ids=list(range(NCORE)),
                               trace=TRACE)
    LAST_RESULT = res

    # ---- host: unshard (accumulate split-window slots) ----
    acc = np.zeros((nwintot * AW, NLAM * NRAD), np.float32)
    for s in range(nslots):
        wid = slot_window[s]
        if wid < 0:
            continue
        part = res.results[s // NWIN]["out"]
        lw = s % NWIN
        acc[wid * AW:(wid + 1) * AW] += part[lw * 128:(lw + 1) * 128]
    out = acc.reshape(nwintot * AW, NLAM, NRAD)[:nat]
    return np.ascontiguousarray(out.transpose(0, 2, 1))   # [nat, NRAD, NLAM]


# revision 3
# speedup vs baseline: 2.3307x; 1.0221x over previous
"""BACENET gnn_message_passing kernel for 8 TRN2 NeuronCores.

Strategy: sort pairs by first_atom_idx on host; atoms grouped into
128-atom windows; each window's pairs padded to a fixed number of
128-pair blocks (nbw, same for every window so all cores run one SPMD
graph).  The scatter one-hot matrices are precomputed on the HOST in
bf16 and DMA'd in (the on-device is_equal build was the bottleneck).
On device, per block: build the angular monomials with wide vector ops
(fp32), form data[p, r*34+l] = radial[p,r]*ang[p,l] with one broadcast
tensor_tensor into a bf16 tile (split across Vector/GpSimd), and
accumulate onehot.T @ data into PSUM with bf16 TensorEngine matmuls
(the segment sum).  Epilogue per window: square (ScalarE), contract
the lambda weights (broadcast multiply + reduce on VectorE), DMA out.
"""

import math
import numpy as np

TRACE = False          # test harness can set kernel.TRACE = True for profiling
LAST_RESULT = None

NAT = 12500
NPAIRS = 250000
NRAD = 16
L = 34
NLAM = 4
NCORE = 8
AW = 128                      # atoms per window
NWINTOT = (NAT + AW - 1) // AW        # 98
NWIN = (NWINTOT + NCORE - 1) // NCORE  # 13 windows per core
FEATC = 20                    # 16 radial + 3 unit vec + 1 local idx


def _graded_order(zeta):
    """Monomial order produced by the on-device recurrence.

    deg1 = [z, y, x]; deg s = [z*deg(s-1)] + [y*(lz==0 sublist of s-1)] + [x*x^(s-1)].
    Returns list of (lx,ly,lz) triples in that order.
    """
    deg = [[(0, 0, 1), (0, 1, 0), (1, 0, 0)]]
    for s in range(2, zeta + 1):
        prev = deg[-1]
        lz0 = [t for t in prev if t[2] == 0]
        cur = ([(a, b, c + 1) for (a, b, c) in prev]
               + [(a, b + 1, 0) for (a, b, _) in lz0]
               + [(prev[-1][0] + 1, 0, 0)])
        cur[-1] = (s, 0, 0)
        deg.append(cur)
    out = []
    for d in deg:
        out.extend(d)
    return out


def _build_graph(nblk, nbw, lx, ly, lz, graded):
    """Build the SPMD Bass graph. Returns nc."""
    import concourse.bass as bass
    import concourse.bacc as bacc
    import concourse.mybir as mybir
    from concourse import tile

    dt = mybir.dt.float32
    bf = mybir.dt.bfloat16
    Alu = mybir.AluOpType
    Act = mybir.ActivationFunctionType

    nc = bacc.Bacc("TRN2", target_bir_lowering=False, debug=False,
                   num_devices=NCORE)

    feat_d = nc.dram_tensor("feat", [128, nblk, FEATC], dt, kind="ExternalInput")
    oh_d = nc.dram_tensor("oh", [128, nblk * 128], bf, kind="ExternalInput")
    w4_d = nc.dram_tensor("w4", [128, NLAM * NRAD * L], bf, kind="ExternalInput")
    out_d = nc.dram_tensor("out", [NWIN * 128, NLAM * NRAD], dt,
                           kind="ExternalOutput")

    W4C = NLAM * NRAD * L  # 2176
    HALF = 8 * L           # 272

    with tile.TileContext(nc) as tc:
        with (
            tc.tile_pool(name="const", bufs=1) as cpool,
            tc.tile_pool(name="work", bufs=2) as pool,
            tc.tile_pool(name="blk", bufs=4) as bpool,
            tc.tile_pool(name="psum", bufs=2, space="PSUM") as psum,
        ):
            w4 = cpool.tile([128, W4C], bf, tag="w4")
            nc.sync.dma_start(w4[:], w4_d[:])

            for w in range(NWIN):
                feat = pool.tile([128, nbw, FEATC], dt, tag="feat")
                nc.sync.dma_start(feat[:], feat_d[:, w * nbw:(w + 1) * nbw, :])
                ohw = pool.tile([128, nbw * 128], bf, tag="ohw")
                nc.sync.dma_start(
                    ohw[:], oh_d[:, w * nbw * 128:(w + 1) * nbw * 128])

                # monomials, layout [128, nbw, L] so l is contiguous
                mono = pool.tile([128, nbw, L], dt, tag="mono")
                for c in range(3):
                    # deg1 order [z, y, x] -> mono cols 0,1,2
                    nc.vector.tensor_scalar(
                        mono[:, :, c], feat[:, :, 18 - c], 1e-12, None, Alu.add)
                if graded:
                    # graded recurrence: 3 contiguous wide ops per degree
                    sizes = [3]
                    zeta = 1
                    while sum(sizes) < L:
                        zeta += 1
                        sizes.append(sizes[-1] + zeta + 1)
                    offs = [0]
                    for sz in sizes[:-1]:
                        offs.append(offs[-1] + sz)
                    for s in range(2, zeta + 1):
                        o_prev, t_prev = offs[s - 2], sizes[s - 2]
                        o_cur = offs[s - 1]
                        uz = feat[:, :, 18].unsqueeze(2).broadcast_to(
                            [128, nbw, t_prev])
                        nc.vector.tensor_tensor(
                            mono[:, :, o_cur:o_cur + t_prev],
                            mono[:, :, o_prev:o_prev + t_prev], uz, Alu.mult)
                        # lz==0 sublist of degree s-1 = last s entries
                        o_lz0 = o_prev + t_prev - s
                        uy = feat[:, :, 17].unsqueeze(2).broadcast_to(
                            [128, nbw, s])
                        nc.vector.tensor_tensor(
                            mono[:, :, o_cur + t_prev:o_cur + t_prev + s],
                            mono[:, :, o_lz0:o_lz0 + s], uy, Alu.mult)
                        ux = feat[:, :, 16].unsqueeze(2).broadcast_to(
                            [128, nbw, 1])
                        nc.vector.tensor_tensor(
                            mono[:, :, o_cur + t_prev + s:o_cur + t_prev + s + 1],
                            mono[:, :, o_prev + t_prev - 1:o_prev + t_prev],
                            ux, Alu.mult)
                else:
                    pow_t = pool.tile([128, 3, 5, nbw], dt, tag="pow")
                    for c, base in ((0, 16), (1, 17), (2, 18)):
                        nc.vector.tensor_scalar(
                            pow_t[:, c, 1, :], feat[:, :, base], 1e-12, None,
                            Alu.add)
                        for e in range(2, 5):
                            nc.vector.tensor_tensor(
                                pow_t[:, c, e, :], pow_t[:, c, e - 1, :],
                                pow_t[:, c, 1, :], Alu.mult)
                    for li in range(L):
                        exps = [(0, lx[li]), (1, ly[li]), (2, lz[li])]
                        exps = [(c, e) for c, e in exps if e > 0]
                        if not exps:
                            nc.vector.memset(mono[:, :, li], 1.0)
                            continue
                        c0, e0 = exps[0]
                        if len(exps) == 1:
                            nc.vector.tensor_copy(mono[:, :, li],
                                                  pow_t[:, c0, e0, :])
                        else:
                            c1, e1 = exps[1]
                            nc.vector.tensor_tensor(
                                mono[:, :, li], pow_t[:, c0, e0, :],
                                pow_t[:, c1, e1, :], Alu.mult)
                            if len(exps) == 3:
                                c2, e2 = exps[2]
                                nc.vector.tensor_tensor(
                                    mono[:, :, li], mono[:, :, li],
                                    pow_t[:, c2, e2, :], Alu.mult)

                # outer product rad x ang for the whole window in two wide
                # TTs (one per engine) to amortize per-op overhead
                dataw = bpool.tile([128, nbw, NRAD, L], bf, tag="dataw")
                spl = (nbw * 13) // 21 or 1
                for lo, hi, eng in ((0, spl, nc.vector),
                                    (spl, nbw, nc.gpsimd)):
                    if lo >= hi:
                        continue
                    nbs = hi - lo
                    radb = feat[:, lo:hi, 0:16].unsqueeze(3).broadcast_to(
                        [128, nbs, NRAD, L])
                    angb = mono[:, lo:hi, :].unsqueeze(2).broadcast_to(
                        [128, nbs, NRAD, L])
                    eng.tensor_tensor(dataw[:, lo:hi], radb, angb, Alu.mult)

                psA = psum.tile([128, HALF], dt, tag="psA")
                psB = psum.tile([128, HALF], dt, tag="psB")
                for b in range(nbw):
                    d2 = dataw[:, b].rearrange("p r l -> p (r l)")
                    ohb = ohw[:, b * 128:(b + 1) * 128]
                    nc.tensor.matmul(psA[:], ohb, d2[:, 0:HALF],
                                     start=(b == 0), stop=(b == nbw - 1))
                    nc.tensor.matmul(psB[:], ohb, d2[:, HALF:2 * HALF],
                                     start=(b == 0), stop=(b == nbw - 1))

                g2 = pool.tile([128, NRAD * L], bf, tag="g2")
                nc.scalar.activation(g2[:, 0:HALF], psA[:], Act.Square)
                nc.scalar.activation(g2[:, HALF:2 * HALF], psB[:], Act.Square)

                prod = pool.tile([128, W4C], bf, tag="prod")
                g2b = g2.unsqueeze(1).broadcast_to([128, NLAM, NRAD * L])
                w4v = w4.rearrange("p (z q) -> p z q", z=NLAM)
                prodv = prod.rearrange("p (z q) -> p z q", z=NLAM)
                nc.vector.tensor_tensor(prodv, g2b, w4v, Alu.mult)

                ow = pool.tile([128, NLAM * NRAD], dt, tag="ow")
                nc.vector.tensor_reduce(
                    ow[:], prod.rearrange("p (q l) -> p q l", l=L),
                    mybir.AxisListType.X, Alu.add)
                nc.sync.dma_start(out_d[w * 128:(w + 1) * 128, :], ow[:])

    return nc


def kernel(**inputs):
    import ml_dtypes

    z = int(inputs["z"])
    rij_unit = np.asarray(inputs["rij_unit"], np.float32)
    radial_ij = np.asarray(inputs["radial_ij"], np.float32)
    first_atom_idx = np.asarray(inputs["first_atom_idx"], np.int32)
    lambda_weights = np.asarray(inputs["lambda_weights"], np.float32)
    lxlylz = np.asarray(inputs["lxlylz"], np.int32)
    lxlylz_sum = np.asarray(inputs["lxlylz_sum"], np.int32)
    fact_norm = np.asarray(inputs["fact_norm"], np.float32)
    nat = int(inputs["nat"])

    npairs = rij_unit.shape[0]
    nwintot = (nat + AW - 1) // AW
    assert nwintot <= NWIN * NCORE

    # ---- host: sort pairs by atom, window them, pack into fixed slots ----
    order = np.argsort(first_atom_idx, kind="stable")
    sidx = first_atom_idx[order]
    wg = sidx // AW                               # window of each sorted pair
    nslots = NWIN * NCORE
    win_counts = np.bincount(wg, minlength=nwintot)
    win_start = np.concatenate([[0], np.cumsum(win_counts)[:-1]])
    bw = (win_counts + 127) // 128                # blocks needed per window
    nbw = max(1, int(np.ceil(bw.sum() / nslots)))
    while int(np.sum(np.maximum((bw + nbw - 1) // nbw, 1))) > nslots:
        nbw += 1
    nblk = NWIN * nbw

    slots_per_win = np.maximum((bw + nbw - 1) // nbw, 1)
    slot0_of_win = np.concatenate([[0], np.cumsum(slots_per_win)[:-1]])
    slot_window = np.full(nslots, -1, np.int64)   # slot -> window id
    for wid in range(nwintot):
        for k in range(slots_per_win[wid]):
            slot_window[slot0_of_win[wid] + k] = wid

    rank = np.arange(npairs) - win_start[wg]      # rank within own window
    slot = slot0_of_win[wg] + rank // (nbw * 128)
    r2 = rank % (nbw * 128)
    nb = r2 // 128
    pp = r2 % 128
    core = slot // NWIN
    col = (slot % NWIN) * nbw + nb

    feat = np.zeros((NCORE, 128, nblk, FEATC), np.float32)
    feat[core, pp, col, 0:16] = radial_ij[order]
    feat[core, pp, col, 16:19] = rij_unit[order]
    lidx = (sidx - wg * AW).astype(np.int64)
    feat[core, pp, col, 19] = lidx.astype(np.float32)

    # host-precomputed scatter one-hot (bf16): oh[p, b*128 + a] = 1 when the
    # pair in partition p of block b belongs to local atom a of its window
    oh = np.zeros((NCORE, 128, nblk * 128), ml_dtypes.bfloat16)
    oh[core, pp, col * 128 + lidx] = 1.0

    # ---- monomial order on device ----
    ltrip = [tuple(t) for t in lxlylz.tolist()]
    graded_ref = _graded_order(4) if len(ltrip) == L else None
    graded = graded_ref is not None and sorted(ltrip) == sorted(graded_ref)
    if graded:
        pos = {}
        for i, t in enumerate(ltrip):
            pos.setdefault(t, []).append(i)
        perm = []
        for t in graded_ref:
            perm.append(pos[t].pop(0))
        perm = np.array(perm, np.int32)           # device l -> input l
    else:
        perm = np.arange(len(ltrip), dtype=np.int32)

    lam = lambda_weights[:, None] ** lxlylz_sum.astype(np.float32)[None, :]
    wrow = lam * fact_norm[None, :] * (2.0 ** (1.0 - float(z)))   # [NLAM, L]
    wrow = wrow[:, perm]                          # reorder to device order
    w4 = np.tile(wrow[:, None, :], (1, NRAD, 1)).reshape(-1)      # (z, r, l)
    w4_t = np.tile(w4[None, :], (128, 1)).astype(ml_dtypes.bfloat16)

    lx, ly, lz_ = (lxlylz[:, 0].tolist(), lxlylz[:, 1].tolist(),
                   lxlylz[:, 2].tolist())

    nc = _build_graph(nblk, nbw, lx, ly, lz_, graded)
    nc.compile()

    from concourse.bass_utils import run_bass_kernel_spmd
    in_maps = [{"feat": feat[i], "oh": oh[i], "w4": w4_t}
               for i in range(NCORE)]
    global LAST_RESULT
    res = run_bass_kernel_spmd(nc, in_maps, core_ids=list(range(NCORE)),
                               trace=TRACE)
    LAST_RESULT = res

    # ---- host: unshard (accumulate split-window slots) ----
    acc = np.zeros((nwintot * AW, NLAM * NRAD), np.float32)
    for s in range(nslots):
        wid = slot_window[s]
        if wid < 0:
            continue
        part = res.results[s // NWIN]["out"]
        lw = s % NWIN
        acc[wid * AW:(wid + 1) * AW] += part[lw * 128:(lw + 1) * 128]
    out = acc.reshape(nwintot * AW, NLAM, NRAD)[:nat]
    return np.ascontiguousarray(out.transpose(0, 2, 1))   # [nat, NRAD, NLAM]
